# revision 1
# baseline (speedup 1.0000x reference)
"""Trainium2 Bass kernel for Clique2NodeConvBasic (GNN message passing).

Computes, for the fixed problem size N=100000 nodes, C=50000 cliques,
E=1600000 edges, D=128:

    gathered = x_clique[clique_idx]            # [E, 128]
    summed   = segment_sum(gathered, node_idx) # [N, 128]
    mean     = summed / max(count, 1)
    out      = mean @ W.T + b                  # [N, 128]

Sharding: edges are partitioned by destination-node range across the 8
NeuronCores (nodes 12500 per core); x_clique and the 128x128 Linear are
replicated. Segment-sum applies locally, no cross-device reduction.

Per-core device algorithm:
  - host sorts edges by destination and buckets them into 98 blocks of
    128 destination nodes; each block's edge list is split by clique id
    at 32768 (dma_gather indices are int16) and padded to a fixed number
    of 128-edge tiles (T_A / T_B, global constants derived from data)
  - dma_gather batches fetch the x_clique rows for a group of blocks
  - a one-hot matrix (edge -> node-within-block) is built with a single
    batched DVE is_equal against an iota tile
  - PE accumulates accum[f, n] += G[e, f].T @ onehot[e, n] in PSUM; the
    gathered tile must be the STATIONARY operand — the PE's moving-operand
    path crashes when streaming a dma_gather-written tile
  - epilogue per block: ACT copies PSUM->SBUF, one matmul with W.T applies
    the Linear directly on the [f, n] accumulator (no transpose needed),
    ACT scales by 1/count (host-precomputed, per-partition = per-node),
    DVE adds the broadcast bias, and the [128, 128] rows are DMA'd out.

Measured on 8 axon NeuronCores: ~1.95-1.97 ms HW exec. The bottleneck is
GpSimd Q7 descriptor generation inside dma_gather (~8 ns per gathered
row, 102%% engine occupancy); HBM, PE, DVE and ACT all run well below
30%% occupancy underneath it.
"""

import os
import sys
import types

sys.path.insert(0, "/opt/trn_rl_repo")

import numpy as np

import concourse.bass as bass
import concourse.mybir as mybir
import concourse.tile as tile
from concourse.vector_clock import ScopedClock, VectorClock
from concourse.bass_utils import run_bass_kernel_spmd

# ----------------------------------------------------------------------------
# Environment shims
# ----------------------------------------------------------------------------

def _install_ntff_shim():
    """Register the axon NTFF profile hook if the image's antenv lacks it."""
    try:
        import antenv
    except ImportError:
        return
    if hasattr(antenv, "axon_hooks"):
        return
    hooks_mod = types.ModuleType("antenv.axon_hooks")
    _store = [None]
    hooks_mod.set_axon_ntff_profile_hook = lambda h: _store.__setitem__(0, h)
    hooks_mod.get_axon_ntff_profile_hook = lambda: _store[0]
    sys.modules["antenv.axon_hooks"] = hooks_mod
    antenv.axon_hooks = hooks_mod
    try:
        from trn_agent_boot.trn_boot import _ntff_profile_via_ctypes

        hook = _ntff_profile_via_ctypes("/opt/axon/libaxon_pjrt.so")
        if hook is not None:
            hooks_mod.set_axon_ntff_profile_hook(hook)
    except Exception:
        pass


_install_ntff_shim()


class PatchedTileContext(tile.TileContext):
    """Spread the tail-drain's sem waits over a chain of SP NOPs.

    The walrus build in this container caps sync-waits per instruction
    (setupSyncWait: "Too many sync wait commands"), while stock Tile
    attaches every outstanding proc's wait to one Drain. One NOP per
    proc keeps every instruction at a single wait.
    """

    def _drain_and_barrier(self, tick_clock, wait_clock):
        gc = tick_clock.global_clock
        for p, t in enumerate(gc):
            if t <= 0:
                continue
            nop = self.nc.sync.nop()
            part = VectorClock()
            part.require_at_least(p, t)
            wait_clock.add_sem_waits(nop.ins, ScopedClock({None: part}))
        self.nc.sync.drain()
        self.nc.all_engine_barrier()
        assert self.sems is not None
        popped = self.nc._tile_sem_poison_stack.pop()
        assert popped is self._sem_poison
        self.nc.clear_and_free_semaphores(list(self.sems.allocated().values()))
        self.nc.all_engine_barrier()


# ----------------------------------------------------------------------------
# Problem constants (hardcoded per the task contract)
# ----------------------------------------------------------------------------

N_NODES = 100000
N_CLIQUES = 50000
D = 128
N_CORES = 8
NPC = N_NODES // N_CORES        # 12500 nodes per core
BLK = 128                       # destination nodes per block
NBLK = -(-NPC // BLK)           # 98 blocks per core (last partial: 84)
NPAD = NBLK * BLK               # 12544 padded output rows per core
SPLIT = 32768                   # int16-index limit for dma_gather
GRP = 2                         # blocks gathered per dma_gather call
PAD_DEST = -1000.0              # one-hot miss value for padding slots

# bf16 halves gather bytes but the kernel is GpSimd-descriptor-bound, so it
# is no faster (1.95ms vs 1.97ms) and costs 4 decades of accuracy. Default f32.
USE_BF16 = os.environ.get("KERNEL_BF16", "0") == "1"

_F32 = mybir.dt.float32
_DT = mybir.dt.bfloat16 if USE_BF16 else _F32
_NP_DT = np.dtype("bfloat16") if False else None  # numpy lacks bf16; use ml_dtypes

if USE_BF16:
    import ml_dtypes

    _NP_DT = np.dtype(ml_dtypes.bfloat16)
else:
    _NP_DT = np.dtype(np.float32)


# ----------------------------------------------------------------------------
# Host-side preparation
# ----------------------------------------------------------------------------

def _prepare(x_clique, node2clique_index):
    """Sort/bucket/pad the edge list. Returns per-core input dicts plus the
    (data-dependent) tile counts T_A, T_B."""
    node = np.asarray(node2clique_index[0]).astype(np.int64)
    clique = np.asarray(node2clique_index[1]).astype(np.int64)

    counts = np.bincount(node, minlength=N_NODES).astype(np.float64)
    inv_cnt = (1.0 / np.maximum(counts, 1.0)).astype(np.float32)

    order = np.argsort(node, kind="stable")
    ns = node[order]
    cs = clique[order]

    core_bounds = np.searchsorted(ns, np.arange(N_CORES + 1) * NPC)

    # First pass: per-(core, block) A/B counts to fix the global T_A, T_B.
    per_core = []
    maxA = 0
    maxB = 0
    for c in range(N_CORES):
        lo, hi = core_bounds[c], core_bounds[c + 1]
        loc = ns[lo:hi] - c * NPC
        cq = cs[lo:hi]
        blk = loc // BLK
        win = loc % BLK
        is_a = cq < SPLIT
        # edges already sorted by loc; stable-partition A before B per block
        key = blk * 2 + (~is_a)
        sub = np.argsort(key, kind="stable")
        blk, win, cq, is_a = blk[sub], win[sub], cq[sub], is_a[sub]
        cntA = np.bincount(blk[is_a], minlength=NBLK)
        cntB = np.bincount(blk[~is_a], minlength=NBLK)
        maxA = max(maxA, int(cntA.max()))
        maxB = max(maxB, int(cntB.max()))
        per_core.append((blk, win, cq, is_a, cntA, cntB))

    T_A = -(-maxA // 128)
    T_B = max(-(-maxB // 128), 1)
    T = T_A + T_B
    LA = T_A * 128
    LB = T_B * 128
    NGRP = -(-NBLK // GRP)      # 49
    np_dt = _NP_DT

    in_maps = []
    for c in range(N_CORES):
        blk, win, cq, is_a, cntA, cntB = per_core[c]

        idxA = np.zeros((NBLK, LA), dtype=np.int16)
        idxB = np.zeros((NBLK, LB), dtype=np.int16)
        dest = np.full((NBLK, T * 128), PAD_DEST, dtype=np.float32)

        offA = np.concatenate([[0], np.cumsum(cntA)])
        offB = np.concatenate([[0], np.cumsum(cntB)])
        nA_tot = int(offA[-1])
        winA = win[: nA_tot] if False else None  # placeholder, replaced below

        # A edges come first within each block (stable partition above)
        a_idx = np.flatnonzero(is_a)
        b_idx = np.flatnonzero(~is_a)
        cqA, winA, blkA = cq[a_idx], win[a_idx], blk[a_idx]
        cqB, winB, blkB = cq[b_idx] - SPLIT, win[b_idx], blk[b_idx]

        posA = np.arange(len(a_idx)) - offA[blkA]
        posB = np.arange(len(b_idx)) - offB[blkB]
        idxA[blkA, posA] = cqA.astype(np.int16)
        idxB[blkB, posB] = cqB.astype(np.int16)
        dest[blkA, posA] = winA
        dest[blkB, posB + LA] = winB


        # wrap indices for dma_gather: seq j -> [j % 16, j // 16], grouped
        # GRP blocks per gather call, concatenated along the free dim.
        # dma_gather reads a [128, n/16] idx AP: the [16, n/16] wrap is
        # replicated across all 8 GpSimd cores' partition groups.
        def _wrap(idx, L):
            w = idx.reshape(NGRP, GRP * L).reshape(NGRP, -1, 16)
            w = np.ascontiguousarray(np.transpose(w, (2, 0, 1))).reshape(16, -1)
            return np.tile(w, (8, 1))

        wA = _wrap(idxA, LA)
        wB = _wrap(idxB, LB)

        # dest layout for the batched is_equal: [128, NBLK * T]
        dest_t = np.ascontiguousarray(
            dest.reshape(NBLK * T, 128).T
        ).astype(np_dt)

        inv_t = np.zeros((BLK, NBLK), dtype=np.float32)
        iv = inv_cnt[c * NPC : (c + 1) * NPC]
        inv_t.T.flat[: NPC] = iv  # row-major [NBLK, BLK] view fill
        inv_t = np.ascontiguousarray(inv_t)

        in_maps.append(
            {
                "idxA": wA,
                "idxB": wB,
                "dest": dest_t,
                "invc": inv_t,
            }
        )

    shared = {
        "xcA": np.ascontiguousarray(np.asarray(x_clique)[:SPLIT]).astype(np_dt),
        "xcB": np.ascontiguousarray(np.asarray(x_clique)[SPLIT:]).astype(np_dt),
        "iota": np.tile(np.arange(128, dtype=np.float32), (128, 1)).astype(np_dt),
    }
    return in_maps, shared, T_A, T_B


# ----------------------------------------------------------------------------
# Kernel builder
# ----------------------------------------------------------------------------

def _build(T_A, T_B):
    T = T_A + T_B
    LA, LB = T_A * 128, T_B * 128
    NGRP = -(-NBLK // GRP)
    CB = N_CLIQUES - SPLIT

    from concourse.bacc import Bacc

    nc = Bacc(None)
    xcA = nc.declare_dram_parameter("xcA", [SPLIT, D], _DT, isOutput=False)
    xcB = nc.declare_dram_parameter("xcB", [CB, D], _DT, isOutput=False)
    idxA = nc.declare_dram_parameter(
        "idxA", [128, NGRP * GRP * LA // 16], mybir.dt.int16, isOutput=False
    )
    idxB = nc.declare_dram_parameter(
        "idxB", [128, NGRP * GRP * LB // 16], mybir.dt.int16, isOutput=False
    )
    dest = nc.declare_dram_parameter("dest", [128, NBLK * T], _DT, isOutput=False)
    invc = nc.declare_dram_parameter("invc", [128, NBLK], _F32, isOutput=False)
    iota = nc.declare_dram_parameter("iota", [128, 128], _DT, isOutput=False)
    wt = nc.declare_dram_parameter("wt", [128, 128], _DT, isOutput=False)
    bb = nc.declare_dram_parameter("bb", [128, 128], _F32, isOutput=False)
    out = nc.declare_dram_parameter("out", [NPAD, D], _F32, isOutput=True)

    from contextlib import ExitStack

    with PatchedTileContext(nc) as tc, ExitStack() as ctx:
        const = ctx.enter_context(tc.tile_pool(name="const", bufs=1))
        sb = ctx.enter_context(tc.tile_pool(name="sb", bufs=3))
        gpool = ctx.enter_context(tc.tile_pool(name="g", bufs=2))
        ps = ctx.enter_context(tc.tile_pool(name="ps", bufs=2, space="PSUM"))

        idxA_t = const.tile([128, NGRP * GRP * LA // 16], mybir.dt.int16)
        nc.sync.dma_start(idxA_t[:], idxA[:])
        idxB_t = const.tile([128, NGRP * GRP * LB // 16], mybir.dt.int16)
        nc.sync.dma_start(idxB_t[:], idxB[:])
        dest_t = const.tile([128, NBLK * T], _DT)
        nc.sync.dma_start(dest_t[:], dest[:])
        invc_t = const.tile([128, NBLK], _F32)
        nc.sync.dma_start(invc_t[:], invc[:])
        iota_t = const.tile([128, 128], _DT)
        nc.sync.dma_start(iota_t[:], iota[:])
        wt_t = const.tile([128, 128], _DT)
        nc.sync.dma_start(wt_t[:], wt[:])
        bb_t = const.tile([128, 128], _F32)
        nc.sync.dma_start(bb_t[:], bb[:])

        nA = GRP * LA
        nB = GRP * LB
        for g in range(NGRP):
            gA = gpool.tile([128, GRP * T_A, 128], _DT, tag="gA")
            nc.gpsimd.dma_gather(
                gA[:],
                xcA[:],
                idxA_t[:, g * (nA // 16) : (g + 1) * (nA // 16)],
                nA,
                nA,
                D,
                single_packet=False,
            )
            gB = gpool.tile([128, GRP * T_B, 128], _DT, tag="gB")
            nc.gpsimd.dma_gather(
                gB[:],
                xcB[:],
                idxB_t[:, g * (nB // 16) : (g + 1) * (nB // 16)],
                nB,
                nB,
                D,
                single_packet=False,
            )
            for i in range(GRP):
                b = g * GRP + i
                if b >= NBLK:
                    break
                onehot = sb.tile([128, T, 128], _DT, tag="oh")
                nc.vector.tensor_tensor(
                    out=onehot[:],
                    in0=dest_t[:, b * T : (b + 1) * T, None].to_broadcast(
                        [128, T, 128]
                    ),
                    in1=iota_t[:, None, :].to_broadcast([128, T, 128]),
                    op=mybir.AluOpType.is_equal,
                )
                # accum[f, n] += G[e, f].T @ onehot[e, n] — the gathered tile
                # must be the STATIONARY operand (LDWEIGHTS path); the moving
                # path crashes the PE when reading a dma_gather-written tile.
                accum = ps.tile([128, 128], _F32, tag="acc")
                for t in range(T_A):
                    nc.tensor.matmul(
                        out=accum[:],
                        lhsT=gA[:, i * T_A + t, :],
                        rhs=onehot[:, t, :],
                        start=(t == 0),
                        stop=False,
                    )
                for t in range(T_B):
                    nc.tensor.matmul(
                        out=accum[:],
                        lhsT=gB[:, i * T_B + t, :],
                        rhs=onehot[:, T_A + t, :],
                        start=False,
                        stop=(t == T_B - 1),
                    )
                # accum is summed.T — exactly the lhsT the Linear wants.
                acc_sb = sb.tile([128, 128], _DT, tag="accsb")
                nc.scalar.activation(
                    acc_sb[:], accum[:], mybir.ActivationFunctionType.Copy
                )
                lin = ps.tile([128, 128], _F32, tag="lin")
                nc.tensor.matmul(
                    out=lin[:], lhsT=acc_sb[:], rhs=wt_t[:], start=True, stop=True
                )
                # out[n, o] = lin[n, o] / count[n] + b[o]
                sc = sb.tile([128, 128], _F32, tag="sc")
                nc.scalar.activation(
                    sc[:],
                    lin[:],
                    mybir.ActivationFunctionType.Copy,
                    scale=invc_t[:, b : b + 1],
                )
                outs = sb.tile([128, 128], _F32, tag="outs")
                nc.vector.tensor_tensor(
                    out=outs[:], in0=sc[:], in1=bb_t[:], op=mybir.AluOpType.add
                )
                nc.sync.dma_start(out[b * 128 : (b + 1) * 128, :], outs[:])

    nc.finalize()
    return nc


_BUILD_CACHE = {}


def kernel(x, x_clique, node2clique_index, W, b, _trace=False, _tmpdir=None):
    in_maps, shared, T_A, T_B = _prepare(x_clique, node2clique_index)

    shared["wt"] = np.ascontiguousarray(np.asarray(W, dtype=np.float32).T).astype(
        _NP_DT
    )
    shared["bb"] = np.tile(
        np.asarray(b, dtype=np.float32)[None, :], (128, 1)
    ).astype(np.float32)

    key = (T_A, T_B, USE_BF16)
    if key not in _BUILD_CACHE:
        _BUILD_CACHE[key] = _build(T_A, T_B)
    nc = _BUILD_CACHE[key]

    full_maps = [dict(m, **shared) for m in in_maps]
    kwargs = {}
    if _trace:
        kwargs = dict(trace=True, tmpdir=_tmpdir)
    res = run_bass_kernel_spmd(nc, full_maps, core_ids=list(range(N_CORES)), **kwargs)

    out = np.concatenate(
        [res.results[c]["out"][:NPC] for c in range(N_CORES)], axis=0
    ).astype(np.float32)
    if _trace:
        return out, res
    return out



# revision 7
# speedup vs baseline: 2.0734x; 2.0734x over previous
"""Trainium2 Bass kernel for Clique2NodeConvBasic (GNN message passing).

Computes, for the fixed problem size N=100000 nodes, C=50000 cliques,
E=1600000 edges, D=128:

    gathered = x_clique[clique_idx]            # [E, 128]
    summed   = segment_sum(gathered, node_idx) # [N, 128]
    mean     = summed / max(count, 1)
    out      = mean @ W.T + b                  # [N, 128]

Sharding: edges are partitioned by destination-node range across the 8
NeuronCores (nodes 12500 per core); x_clique and the 128x128 Linear are
replicated. Segment-sum applies locally, no cross-device reduction.

Per-core device algorithm:
  - host sorts edges by destination and buckets them into 98 blocks of
    128 destination nodes; each block's edge list is split by clique id
    at 32768 (dma_gather indices are int16) and padded to a fixed number
    of 128-edge tiles (T_A / T_B, global constants derived from data)
  - dma_gather batches fetch the x_clique rows for a group of blocks
  - a one-hot matrix (edge -> node-within-block) is built with a single
    batched DVE is_equal against an iota tile
  - PE accumulates accum[f, n] += G[e, f].T @ onehot[e, n] in PSUM; the
    gathered tile must be the STATIONARY operand — the PE's moving-operand
    path crashes when streaming a dma_gather-written tile
  - epilogue per block: ACT copies PSUM->SBUF, one matmul with W.T applies
    the Linear directly on the [f, n] accumulator (no transpose needed),
    ACT scales by 1/count (host-precomputed, per-partition = per-node),
    DVE adds the broadcast bias, and the [128, 128] rows are DMA'd out.

Measured on 8 axon NeuronCores: ~1.95-1.97 ms HW exec. The bottleneck is
GpSimd Q7 descriptor generation inside dma_gather (~8 ns per gathered
row, 102%% engine occupancy); HBM, PE, DVE and ACT all run well below
30%% occupancy underneath it.
"""

import os
import sys
import types

sys.path.insert(0, "/opt/trn_rl_repo")

import numpy as np

import concourse.bass as bass
import concourse.mybir as mybir
import concourse.tile as tile
from concourse.vector_clock import ScopedClock, VectorClock
from concourse.bass_utils import run_bass_kernel_spmd

# ----------------------------------------------------------------------------
# Environment shims
# ----------------------------------------------------------------------------

def _install_ntff_shim():
    """Register the axon NTFF profile hook if the image's antenv lacks it."""
    try:
        import antenv
    except ImportError:
        return
    if hasattr(antenv, "axon_hooks"):
        return
    hooks_mod = types.ModuleType("antenv.axon_hooks")
    _store = [None]
    hooks_mod.set_axon_ntff_profile_hook = lambda h: _store.__setitem__(0, h)
    hooks_mod.get_axon_ntff_profile_hook = lambda: _store[0]
    sys.modules["antenv.axon_hooks"] = hooks_mod
    antenv.axon_hooks = hooks_mod
    try:
        from trn_agent_boot.trn_boot import _ntff_profile_via_ctypes

        hook = _ntff_profile_via_ctypes("/opt/axon/libaxon_pjrt.so")
        if hook is not None:
            hooks_mod.set_axon_ntff_profile_hook(hook)
    except Exception:
        pass


_install_ntff_shim()


class PatchedTileContext(tile.TileContext):
    """Spread the tail-drain's sem waits over a chain of SP NOPs.

    The walrus build in this container caps sync-waits per instruction
    (setupSyncWait: "Too many sync wait commands"), while stock Tile
    attaches every outstanding proc's wait to one Drain. One NOP per
    proc keeps every instruction at a single wait.
    """

    def _drain_and_barrier(self, tick_clock, wait_clock):
        gc = tick_clock.global_clock
        for p, t in enumerate(gc):
            if t <= 0:
                continue
            nop = self.nc.sync.nop()
            part = VectorClock()
            part.require_at_least(p, t)
            wait_clock.add_sem_waits(nop.ins, ScopedClock({None: part}))
        self.nc.sync.drain()
        self.nc.all_engine_barrier()
        assert self.sems is not None
        popped = self.nc._tile_sem_poison_stack.pop()
        assert popped is self._sem_poison
        self.nc.clear_and_free_semaphores(list(self.sems.allocated().values()))
        self.nc.all_engine_barrier()


# ----------------------------------------------------------------------------
# Problem constants (hardcoded per the task contract)
# ----------------------------------------------------------------------------

N_NODES = 100000
N_CLIQUES = 50000
D = 128
N_CORES = 8
NPC = N_NODES // N_CORES        # 12500 nodes per core
BLK = 128                       # destination nodes per block
NBLK = -(-NPC // BLK)           # 98 blocks per core (last partial: 84)
NPAD = NBLK * BLK               # 12544 padded output rows per core
SPLIT = 32768                   # int16-index limit for dma_gather
GRP = 2                         # blocks gathered per dma_gather call
PAD_DEST = -1000.0              # one-hot miss value for padding slots

# bf16 halves gather bytes but the kernel is GpSimd-descriptor-bound, so it
# is no faster (1.95ms vs 1.97ms) and costs 4 decades of accuracy. Default f32.
USE_BF16 = os.environ.get("KERNEL_BF16", "0") == "1"

# SWDGE queues: each dma_gather runs on GpSimd core pair (2q, 2q+1); with
# NQ>1 consecutive gathers go to different pairs and may overlap on HW.
NQ = int(os.environ.get("KERNEL_NQ", "1"))

_F32 = mybir.dt.float32
_DT = mybir.dt.bfloat16 if USE_BF16 else _F32
_NP_DT = np.dtype("bfloat16") if False else None  # numpy lacks bf16; use ml_dtypes

if USE_BF16:
    import ml_dtypes

    _NP_DT = np.dtype(ml_dtypes.bfloat16)
else:
    _NP_DT = np.dtype(np.float32)


# ----------------------------------------------------------------------------
# Host-side preparation
# ----------------------------------------------------------------------------

def _prepare(x_clique, node2clique_index):
    """Sort/bucket/pad the edge list. Returns per-core input dicts plus the
    (data-dependent) tile counts T_A, T_B."""
    node = np.asarray(node2clique_index[0]).astype(np.int64)
    clique = np.asarray(node2clique_index[1]).astype(np.int64)

    counts = np.bincount(node, minlength=N_NODES).astype(np.float64)
    inv_cnt = (1.0 / np.maximum(counts, 1.0)).astype(np.float32)

    order = np.argsort(node, kind="stable")
    ns = node[order]
    cs = clique[order]

    core_bounds = np.searchsorted(ns, np.arange(N_CORES + 1) * NPC)

    # First pass: per-(core, block) A/B counts to fix the global T_A, T_B.
    per_core = []
    maxA = 0
    maxB = 0
    for c in range(N_CORES):
        lo, hi = core_bounds[c], core_bounds[c + 1]
        loc = ns[lo:hi] - c * NPC
        cq = cs[lo:hi]
        blk = loc // BLK
        win = loc % BLK
        is_a = cq < SPLIT
        # edges already sorted by loc; stable-partition A before B per block
        key = blk * 2 + (~is_a)
        sub = np.argsort(key, kind="stable")
        blk, win, cq, is_a = blk[sub], win[sub], cq[sub], is_a[sub]
        cntA = np.bincount(blk[is_a], minlength=NBLK)
        cntB = np.bincount(blk[~is_a], minlength=NBLK)
        maxA = max(maxA, int(cntA.max()))
        maxB = max(maxB, int(cntB.max()))
        per_core.append((blk, win, cq, is_a, cntA, cntB))

    T_A = -(-maxA // 128)
    T_B = max(-(-maxB // 128), 1)
    T = T_A + T_B
    LA = T_A * 128
    LB = T_B * 128
    NGRP = -(-NBLK // GRP)      # 49
    np_dt = _NP_DT

    in_maps = []
    for c in range(N_CORES):
        blk, win, cq, is_a, cntA, cntB = per_core[c]

        idxA = np.zeros((NBLK, LA), dtype=np.int16)
        idxB = np.zeros((NBLK, LB), dtype=np.int16)
        dest = np.full((NBLK, T * 128), PAD_DEST, dtype=np.float32)

        offA = np.concatenate([[0], np.cumsum(cntA)])
        offB = np.concatenate([[0], np.cumsum(cntB)])
        nA_tot = int(offA[-1])
        winA = win[: nA_tot] if False else None  # placeholder, replaced below

        # A edges come first within each block (stable partition above)
        a_idx = np.flatnonzero(is_a)
        b_idx = np.flatnonzero(~is_a)
        cqA, winA, blkA = cq[a_idx], win[a_idx], blk[a_idx]
        cqB, winB, blkB = cq[b_idx] - SPLIT, win[b_idx], blk[b_idx]

        posA = np.arange(len(a_idx)) - offA[blkA]
        posB = np.arange(len(b_idx)) - offB[blkB]
        idxA[blkA, posA] = cqA.astype(np.int16)
        idxB[blkB, posB] = cqB.astype(np.int16)
        dest[blkA, posA] = winA
        dest[blkB, posB + LA] = winB


        # wrap indices for dma_gather: seq j -> [j % 16, j // 16], grouped
        # GRP blocks per gather call, concatenated along the free dim.
        # dma_gather reads a [128, n/16] idx AP: the [16, n/16] wrap is
        # replicated across all 8 GpSimd cores' partition groups.
        def _wrap(idx, L):
            w = idx.reshape(NGRP, GRP * L).reshape(NGRP, -1, 16)
            w = np.ascontiguousarray(np.transpose(w, (2, 0, 1))).reshape(16, -1)
            return np.tile(w, (8, 1))

        wA = _wrap(idxA, LA)
        wB = _wrap(idxB, LB)

        # dest layout for the batched is_equal: [128, NBLK * T]
        dest_t = np.ascontiguousarray(
            dest.reshape(NBLK * T, 128).T
        ).astype(np_dt)

        inv_t = np.zeros((BLK, NBLK), dtype=np.float32)
        iv = inv_cnt[c * NPC : (c + 1) * NPC]
        inv_t.T.flat[: NPC] = iv  # row-major [NBLK, BLK] view fill
        inv_t = np.ascontiguousarray(inv_t)

        in_maps.append(
            {
                "idxA": wA,
                "idxB": wB,
                "dest": dest_t,
                "invc": inv_t,
            }
        )

    shared = {
        "xcA": np.ascontiguousarray(np.asarray(x_clique)[:SPLIT]).astype(np_dt),
        "xcB": np.ascontiguousarray(np.asarray(x_clique)[SPLIT:]).astype(np_dt),
        "iota": np.tile(np.arange(128, dtype=np.float32), (128, 1)).astype(np_dt),
    }
    return in_maps, shared, T_A, T_B


# ----------------------------------------------------------------------------
# Kernel builder
# ----------------------------------------------------------------------------

def _build(T_A, T_B):
    T = T_A + T_B
    LA, LB = T_A * 128, T_B * 128
    NGRP = -(-NBLK // GRP)
    CB = N_CLIQUES - SPLIT

    from concourse.bacc import Bacc

    nc = Bacc(None, num_swdge_queues=NQ)
    xcA = nc.declare_dram_parameter("xcA", [SPLIT, D], _DT, isOutput=False)
    xcB = nc.declare_dram_parameter("xcB", [CB, D], _DT, isOutput=False)
    idxA = nc.declare_dram_parameter(
        "idxA", [128, NGRP * GRP * LA // 16], mybir.dt.int16, isOutput=False
    )
    idxB = nc.declare_dram_parameter(
        "idxB", [128, NGRP * GRP * LB // 16], mybir.dt.int16, isOutput=False
    )
    dest = nc.declare_dram_parameter("dest", [128, NBLK * T], _DT, isOutput=False)
    invc = nc.declare_dram_parameter("invc", [128, NBLK], _F32, isOutput=False)
    iota = nc.declare_dram_parameter("iota", [128, 128], _DT, isOutput=False)
    wt = nc.declare_dram_parameter("wt", [128, 128], _DT, isOutput=False)
    bb = nc.declare_dram_parameter("bb", [128, 128], _F32, isOutput=False)
    out = nc.declare_dram_parameter("out", [NPAD, D], _F32, isOutput=True)

    from contextlib import ExitStack

    with PatchedTileContext(nc) as tc, ExitStack() as ctx:
        const = ctx.enter_context(tc.tile_pool(name="const", bufs=1))
        sb = ctx.enter_context(tc.tile_pool(name="sb", bufs=3))
        gpool = ctx.enter_context(tc.tile_pool(name="g", bufs=max(2, NQ - 1)))
        ps = ctx.enter_context(tc.tile_pool(name="ps", bufs=2, space="PSUM"))

        idxA_t = const.tile([128, NGRP * GRP * LA // 16], mybir.dt.int16)
        nc.sync.dma_start(idxA_t[:], idxA[:])
        idxB_t = const.tile([128, NGRP * GRP * LB // 16], mybir.dt.int16)
        nc.sync.dma_start(idxB_t[:], idxB[:])
        dest_t = const.tile([128, NBLK * T], _DT)
        nc.sync.dma_start(dest_t[:], dest[:])
        invc_t = const.tile([128, NBLK], _F32)
        nc.sync.dma_start(invc_t[:], invc[:])
        iota_t = const.tile([128, 128], _DT)
        nc.sync.dma_start(iota_t[:], iota[:])
        wt_t = const.tile([128, 128], _DT)
        nc.sync.dma_start(wt_t[:], wt[:])
        bb_t = const.tile([128, 128], _F32)
        nc.sync.dma_start(bb_t[:], bb[:])

        nA = GRP * LA
        nB = GRP * LB
        for g in range(NGRP):
            gA = gpool.tile([128, GRP * T_A, 128], _DT, tag="gA")
            nc.gpsimd.dma_gather(
                gA[:],
                xcA[:],
                idxA_t[:, g * (nA // 16) : (g + 1) * (nA // 16)],
                nA,
                nA,
                D,
                single_packet=False,
                queue_num=(2 * g) % NQ,
            )
            gB = gpool.tile([128, GRP * T_B, 128], _DT, tag="gB")
            nc.gpsimd.dma_gather(
                gB[:],
                xcB[:],
                idxB_t[:, g * (nB // 16) : (g + 1) * (nB // 16)],
                nB,
                nB,
                D,
                single_packet=False,
                queue_num=(2 * g + 1) % NQ,
            )
            for i in range(GRP):
                b = g * GRP + i
                if b >= NBLK:
                    break
                onehot = sb.tile([128, T, 128], _DT, tag="oh")
                nc.vector.tensor_tensor(
                    out=onehot[:],
                    in0=dest_t[:, b * T : (b + 1) * T, None].to_broadcast(
                        [128, T, 128]
                    ),
                    in1=iota_t[:, None, :].to_broadcast([128, T, 128]),
                    op=mybir.AluOpType.is_equal,
                )
                # accum[f, n] += G[e, f].T @ onehot[e, n] — the gathered tile
                # must be the STATIONARY operand (LDWEIGHTS path); the moving
                # path crashes the PE when reading a dma_gather-written tile.
                accum = ps.tile([128, 128], _F32, tag="acc")
                for t in range(T_A):
                    nc.tensor.matmul(
                        out=accum[:],
                        lhsT=gA[:, i * T_A + t, :],
                        rhs=onehot[:, t, :],
                        start=(t == 0),
                        stop=False,
                    )
                for t in range(T_B):
                    nc.tensor.matmul(
                        out=accum[:],
                        lhsT=gB[:, i * T_B + t, :],
                        rhs=onehot[:, T_A + t, :],
                        start=False,
                        stop=(t == T_B - 1),
                    )
                # accum is summed.T — exactly the lhsT the Linear wants.
                acc_sb = sb.tile([128, 128], _DT, tag="accsb")
                nc.scalar.activation(
                    acc_sb[:], accum[:], mybir.ActivationFunctionType.Copy
                )
                lin = ps.tile([128, 128], _F32, tag="lin")
                nc.tensor.matmul(
                    out=lin[:], lhsT=acc_sb[:], rhs=wt_t[:], start=True, stop=True
                )
                # out[n, o] = lin[n, o] / count[n] + b[o]
                sc = sb.tile([128, 128], _F32, tag="sc")
                nc.scalar.activation(
                    sc[:],
                    lin[:],
                    mybir.ActivationFunctionType.Copy,
                    scale=invc_t[:, b : b + 1],
                )
                outs = sb.tile([128, 128], _F32, tag="outs")
                nc.vector.tensor_tensor(
                    out=outs[:], in0=sc[:], in1=bb_t[:], op=mybir.AluOpType.add
                )
                nc.sync.dma_start(out[b * 128 : (b + 1) * 128, :], outs[:])

    nc.finalize()
    return nc


_BUILD_CACHE = {}


def kernel(x, x_clique, node2clique_index, W, b, _trace=False, _tmpdir=None):
    in_maps, shared, T_A, T_B = _prepare(x_clique, node2clique_index)

    shared["wt"] = np.ascontiguousarray(np.asarray(W, dtype=np.float32).T).astype(
        _NP_DT
    )
    shared["bb"] = np.tile(
        np.asarray(b, dtype=np.float32)[None, :], (128, 1)
    ).astype(np.float32)

    key = (T_A, T_B, USE_BF16, NQ)
    if key not in _BUILD_CACHE:
        _BUILD_CACHE[key] = _build(T_A, T_B)
    nc = _BUILD_CACHE[key]

    full_maps = [dict(m, **shared) for m in in_maps]
    kwargs = {}
    if _trace:
        kwargs = dict(trace=True, tmpdir=_tmpdir)
    res = run_bass_kernel_spmd(nc, full_maps, core_ids=list(range(N_CORES)), **kwargs)

    out = np.concatenate(
        [res.results[c]["out"][:NPC] for c in range(N_CORES)], axis=0
    ).astype(np.float32)
    if _trace:
        return out, res
    return out

